# revision 6
# baseline (speedup 1.0000x reference)
"""Trainium2 Bass kernel for nn_EntAttr_Model (embedding lookup + vocab-parallel linear).

Computation (see reference):
    ent_idx, rel_idx = batch_data[:, 0], batch_data[:, 1]          # [B]
    w_idx = ent2word[ent_idx]                                      # [B, 2]
    m_t = concat([words[w_idx[:,0]], words[w_idx[:,1]], rel[rel_idx]], axis=1)  # [B, 3H]
    out = m_t @ W.T + b                                            # [B, NUM_ENT]

Sharding: W/b/out split along NUM_ENT across 8 cores (vocab parallel); the
small tables + indices are replicated. Each core runs the same program:
  - two-level embedding gather via indirect DMA (SWDGE)
  - PE-transpose of the gathered features into lhsT layout [3H, B]
  - float32r matmuls streaming the (host-pre-transposed) W shard [3H, E]
  - bias add fused into the PSUM->SBUF copy, DMA out [B, E_shard]
Host side only shards/transposes/concats numpy arrays.
"""
import sys

if "/opt/trn_rl_repo" not in sys.path:
    sys.path.insert(0, "/opt/trn_rl_repo")

import numpy as np

import concourse.bass as bass
import concourse.bacc as bacc
import concourse.mybir as mybir
import concourse.tile as tile
from concourse import library_config
from concourse.bass import IndirectOffsetOnAxis
from concourse.bass_utils import run_bass_kernel_spmd
from concourse.masks import make_identity

P = 128
BATCH = 1024
HIDDEN = 256
K = 3 * HIDDEN            # 768 contraction dim
NUM_ENT = 100000
WORD_NUM = 50000
NUM_REL = 500
N_CORES = 8
E_SHARD = NUM_ENT // N_CORES      # 12500
N_TILE = 512
N_TILES = (E_SHARD + N_TILE - 1) // N_TILE   # 25 (last one ragged: 212 cols)
E_PAD = N_TILES * N_TILE          # 12800
B_TILES = BATCH // P              # 8
K_TILES = K // P                  # 6

F32 = mybir.dt.float32
F32R = mybir.dt.float32r
I32 = mybir.dt.int32


def build_nc() -> bass.Bass:
    nc = bacc.Bacc(None)

    bd = nc.declare_dram_parameter("bd", [BATCH, 2], I32, isOutput=False)
    e2w = nc.declare_dram_parameter("e2w", [NUM_ENT, 2], I32, isOutput=False)
    words = nc.declare_dram_parameter("words", [WORD_NUM, HIDDEN], F32, isOutput=False)
    rels = nc.declare_dram_parameter("rels", [NUM_REL, HIDDEN], F32, isOutput=False)
    wt = nc.declare_dram_parameter("wt", [K, E_PAD], F32R, isOutput=False)
    bias = nc.declare_dram_parameter("bias", [1, E_PAD], F32, isOutput=False)
    out = nc.declare_dram_parameter("out", [BATCH, E_SHARD], F32, isOutput=True)

    wt3 = wt.rearrange("(kb p) e -> p kb e", p=P)  # [128, 6, 12800]

    with tile.TileContext(nc) as tc:
        with (
            tc.tile_pool(name="persist", bufs=1) as persist,
            tc.tile_pool(name="gat", bufs=3) as gat,
            tc.tile_pool(name="wp", bufs=3) as wp,
            tc.tile_pool(name="op", bufs=12) as op,
            tc.tile_pool(name="ps", bufs=8, space="PSUM") as ps,
        ):
            # ---- prefix: identity, bias row, gathers, transposes ----
            ident = persist.tile([P, P], F32, tag="ident")
            make_identity(nc, ident[:])
            nc.gpsimd.load_library(library_config.mlp)

            bias_sb = persist.tile([P, E_PAD], F32, tag="bias")
            nc.sync.dma_start(out=bias_sb[:1, :], in_=bias[:])

            # m_T[bt][kb]: [128 k, 128 b] stationary tiles (distinct tags -> exact deps)
            m_T = [
                [
                    persist.tile([P, P], F32R, tag=f"mt{bt}_{kb}", name=f"mt{bt}_{kb}")
                    for kb in range(K_TILES)
                ]
                for bt in range(B_TILES)
            ]

            for bt in range(B_TILES):
                bd_sb = gat.tile([P, 2], I32, tag="bd")
                nc.sync.dma_start(out=bd_sb[:], in_=bd[bt * P:(bt + 1) * P, :])
                widx = gat.tile([P, 2], I32, tag="widx")
                nc.gpsimd.indirect_dma_start(
                    out=widx[:],
                    out_offset=None,
                    in_=e2w[:],
                    in_offset=IndirectOffsetOnAxis(ap=bd_sb[:, 0:1], axis=0),
                )
                m_sb = gat.tile([P, K], F32, tag="m")
                # one indirect DMA per index column: a [128,2] offset AP sims
                # correctly but only the first column takes effect on HW
                nc.gpsimd.indirect_dma_start(
                    out=m_sb[:, 0:HIDDEN],
                    out_offset=None,
                    in_=words[:],
                    in_offset=IndirectOffsetOnAxis(ap=widx[:, 0:1], axis=0),
                )
                nc.gpsimd.indirect_dma_start(
                    out=m_sb[:, HIDDEN:2 * HIDDEN],
                    out_offset=None,
                    in_=words[:],
                    in_offset=IndirectOffsetOnAxis(ap=widx[:, 1:2], axis=0),
                )
                nc.gpsimd.indirect_dma_start(
                    out=m_sb[:, 2 * HIDDEN:K],
                    out_offset=None,
                    in_=rels[:],
                    in_offset=IndirectOffsetOnAxis(ap=bd_sb[:, 1:2], axis=0),
                )
                for kb in range(K_TILES):
                    pt = ps.tile([P, P], F32, tag="ps")
                    nc.tensor.transpose(
                        out=pt[:], in_=m_sb[:, kb * P:(kb + 1) * P], identity=ident[:]
                    )
                    nc.vector.tensor_copy(out=m_T[bt][kb][:], in_=pt[:])

            # bias broadcast, chunked per N-tile so the first chunks are ready early
            for n in range(N_TILES):
                sl = slice(n * N_TILE, (n + 1) * N_TILE)
                nc.gpsimd.partition_broadcast(bias_sb[:, sl], bias_sb[:1, sl])

            # ---- main loop: stream W shard once ----
            for n in range(N_TILES):
                cols = min(N_TILE, E_SHARD - n * N_TILE)
                wsb = wp.tile([P, K_TILES, N_TILE], F32R, tag="w")
                nc.sync.dma_start(
                    out=wsb[:], in_=wt3[:, :, n * N_TILE:(n + 1) * N_TILE]
                )
                for bt in range(B_TILES):
                    acc = ps.tile([P, N_TILE], F32, tag="ps")
                    for kb in range(K_TILES):
                        nc.tensor.matmul(
                            out=acc[:],
                            lhsT=m_T[bt][kb][:],
                            rhs=wsb[:, kb, :],
                            start=(kb == 0),
                            stop=(kb == K_TILES - 1),
                        )
                    osb = op.tile([P, N_TILE], F32, tag="o")
                    nc.vector.tensor_tensor(
                        out=osb[:],
                        in0=acc[:],
                        in1=bias_sb[:, n * N_TILE:(n + 1) * N_TILE],
                        op=mybir.AluOpType.add,
                    )
                    nc.scalar.dma_start(
                        out=out[bt * P:(bt + 1) * P, n * N_TILE:n * N_TILE + cols],
                        in_=osb[:, :cols],
                    )
    nc.compile()
    return nc


_NC = None


def _get_nc():
    global _NC
    if _NC is None:
        _NC = build_nc()
    return _NC


def _np_to_fp32r(x: np.ndarray) -> np.ndarray:
    """Round fp32 -> fp32r encoding (IEEE bits with mantissa RTN-rounded to 11
    explicit bits, low 12 bits zero). Matches walrus cast_fp32_to_fp32r exactly.
    The PE consumes fp32r at full rate; raw IEEE bits in an fp32r operand are
    misinterpreted, so the encode must happen before upload."""
    u = np.ascontiguousarray(x, np.float32).view(np.uint32)
    keep = (u >> 12).astype(np.uint32)
    rbit = (u >> 11) & np.uint32(1)
    sticky = (u & np.uint32(0x7FF)) != 0
    inc = (rbit & (sticky.astype(np.uint32) | (keep & np.uint32(1)))).astype(np.uint32)
    return (((keep + inc) << np.uint32(12)).astype(np.uint32)).view(np.float32)


def _make_in_maps(batch_data, ent2word, words_embd, rel_embed, W, b):
    bd32 = np.ascontiguousarray(np.asarray(batch_data).astype(np.int32))
    e2w = np.ascontiguousarray(np.asarray(ent2word, dtype=np.int32))
    words = np.ascontiguousarray(np.asarray(words_embd, dtype=np.float32))
    rels = np.ascontiguousarray(np.asarray(rel_embed, dtype=np.float32))
    W = np.asarray(W, dtype=np.float32)
    b = np.asarray(b, dtype=np.float32)

    in_maps = []
    for c in range(N_CORES):
        sl = slice(c * E_SHARD, (c + 1) * E_SHARD)
        wt = np.zeros((K, E_PAD), np.float32)
        wt[:, :E_SHARD] = W[sl].T
        wt = _np_to_fp32r(wt)
        bias = np.zeros((1, E_PAD), np.float32)
        bias[0, :E_SHARD] = b[sl]
        in_maps.append(
            {"bd": bd32, "e2w": e2w, "words": words, "rels": rels,
             "wt": wt, "bias": bias}
        )
    return in_maps


def run(trace=False, **inputs):
    nc = _get_nc()
    in_maps = _make_in_maps(**inputs)
    res = run_bass_kernel_spmd(nc, in_maps, core_ids=list(range(N_CORES)), trace=trace)
    out = np.concatenate([r["out"] for r in res.results], axis=1)
    return out, res


def kernel(**inputs) -> np.ndarray:
    out, _ = run(trace=False, **inputs)
    return out


# revision 7
# speedup vs baseline: 1.1429x; 1.1429x over previous
"""Trainium2 Bass kernel for nn_EntAttr_Model (embedding lookup + vocab-parallel linear).

Computation (see reference):
    ent_idx, rel_idx = batch_data[:, 0], batch_data[:, 1]          # [B]
    w_idx = ent2word[ent_idx]                                      # [B, 2]
    m_t = concat([words[w_idx[:,0]], words[w_idx[:,1]], rel[rel_idx]], axis=1)  # [B, 3H]
    out = m_t @ W.T + b                                            # [B, NUM_ENT]

Sharding: W/b/out split along NUM_ENT across 8 cores (vocab parallel); the
small tables + indices are replicated. Each core runs the same program:
  - two-level embedding gather via indirect DMA (SWDGE)
  - PE-transpose of the gathered features into lhsT layout [3H, B]
  - matmuls streaming the (host-pre-transposed) W shard [3H, E] once
  - bias add fused into the PSUM->SBUF copy, DMA out [B, E_shard]
Host side only shards/transposes/concats numpy arrays.

COMPUTE_DTYPE selects the TensorEngine dtype for W and the m_t tiles:
  - "f32r": fp32r (fp32 with 11-bit mantissa; walrus-compatible host encode)
  - "bf16": bfloat16
PSUM accumulation is fp32 either way; bias + output stay fp32.
"""
import os
import sys

if "/opt/trn_rl_repo" not in sys.path:
    sys.path.insert(0, "/opt/trn_rl_repo")

import ml_dtypes
import numpy as np

import concourse.bass as bass
import concourse.bacc as bacc
import concourse.mybir as mybir
import concourse.tile as tile
from concourse.bass import IndirectOffsetOnAxis
from concourse.bass_utils import run_bass_kernel_spmd
from concourse.masks import make_identity

P = 128
BATCH = 1024
HIDDEN = 256
K = 3 * HIDDEN            # 768 contraction dim
NUM_ENT = 100000
WORD_NUM = 50000
NUM_REL = 500
N_CORES = 8
E_SHARD = NUM_ENT // N_CORES      # 12500
N_TILE = 512
N_TILES = (E_SHARD + N_TILE - 1) // N_TILE   # 25 (last one ragged: 212 cols)
E_PAD = N_TILES * N_TILE          # 12800
B_TILES = BATCH // P              # 8
K_TILES = K // P                  # 6

COMPUTE_DTYPE = os.environ.get("BASS_KERNEL_DTYPE", "f32r")

F32 = mybir.dt.float32
I32 = mybir.dt.int32
MM_DT = mybir.dt.float32r if COMPUTE_DTYPE == "f32r" else mybir.dt.bfloat16


def build_nc() -> bass.Bass:
    nc = bacc.Bacc(None)

    bd = nc.declare_dram_parameter("bd", [BATCH, 2], I32, isOutput=False)
    e2w = nc.declare_dram_parameter("e2w", [NUM_ENT, 2], I32, isOutput=False)
    words = nc.declare_dram_parameter("words", [WORD_NUM, HIDDEN], F32, isOutput=False)
    rels = nc.declare_dram_parameter("rels", [NUM_REL, HIDDEN], F32, isOutput=False)
    wt = nc.declare_dram_parameter("wt", [K, E_PAD], MM_DT, isOutput=False)
    bias = nc.declare_dram_parameter("bias", [1, E_PAD], F32, isOutput=False)
    out = nc.declare_dram_parameter("out", [BATCH, E_SHARD], F32, isOutput=True)

    wt3 = wt.rearrange("(kb p) e -> p kb e", p=P)  # [128, 6, 12800]

    with tile.TileContext(nc) as tc:
        with (
            tc.tile_pool(name="persist", bufs=1) as persist,
            tc.tile_pool(name="gat", bufs=4) as gat,
            tc.tile_pool(name="wp", bufs=4) as wp,
            tc.tile_pool(name="op", bufs=12) as op,
            tc.tile_pool(name="ps", bufs=8, space="PSUM") as ps,
        ):
            # ---- prefix ----
            ident = persist.tile([P, P], F32, tag="ident")
            make_identity(nc, ident[:])

            bias_sb = persist.tile([P, E_PAD], F32, tag="bias")

            m_T = [
                [
                    persist.tile([P, P], MM_DT, tag=f"mt{bt}_{kb}", name=f"mt{bt}_{kb}")
                    for kb in range(K_TILES)
                ]
                for bt in range(B_TILES)
            ]

            # index loads + first-level gather for every b-tile up front, so the
            # second-level gathers never stall the SWDGE descriptor pipeline
            bd_sbs, widxs = [], []
            for bt in range(B_TILES):
                bd_sb = gat.tile([P, 2], I32, tag=f"bd{bt}", name=f"bd{bt}")
                nc.sync.dma_start(out=bd_sb[:], in_=bd[bt * P:(bt + 1) * P, :])
                bd_sbs.append(bd_sb)
            for bt in range(B_TILES):
                widx = gat.tile([P, 2], I32, tag=f"wi{bt}", name=f"wi{bt}")
                nc.gpsimd.indirect_dma_start(
                    out=widx[:],
                    out_offset=None,
                    in_=e2w[:],
                    in_offset=IndirectOffsetOnAxis(ap=bd_sbs[bt][:, 0:1], axis=0),
                )
                widxs.append(widx)

            for bt in range(B_TILES):
                m_sb = gat.tile([P, K], F32, tag="m")
                # one indirect DMA per index column: a [128,2] offset AP sims
                # correctly but only the first column takes effect on HW
                nc.gpsimd.indirect_dma_start(
                    out=m_sb[:, 0:HIDDEN],
                    out_offset=None,
                    in_=words[:],
                    in_offset=IndirectOffsetOnAxis(ap=widxs[bt][:, 0:1], axis=0),
                )
                nc.gpsimd.indirect_dma_start(
                    out=m_sb[:, HIDDEN:2 * HIDDEN],
                    out_offset=None,
                    in_=words[:],
                    in_offset=IndirectOffsetOnAxis(ap=widxs[bt][:, 1:2], axis=0),
                )
                nc.gpsimd.indirect_dma_start(
                    out=m_sb[:, 2 * HIDDEN:K],
                    out_offset=None,
                    in_=rels[:],
                    in_offset=IndirectOffsetOnAxis(ap=bd_sbs[bt][:, 1:2], axis=0),
                )
                for kb in range(K_TILES):
                    pt = ps.tile([P, P], F32, tag="ps")
                    nc.tensor.transpose(
                        out=pt[:], in_=m_sb[:, kb * P:(kb + 1) * P], identity=ident[:]
                    )
                    nc.vector.tensor_copy(out=m_T[bt][kb][:], in_=pt[:])

            # ---- main loop: stream W shard once ----
            for n in range(N_TILES):
                cols = min(N_TILE, E_SHARD - n * N_TILE)
                nsl = slice(n * N_TILE, (n + 1) * N_TILE)
                wsb = wp.tile([P, K_TILES, N_TILE], MM_DT, tag="w")
                nc.sync.dma_start(out=wsb[:], in_=wt3[:, :, nsl])
                # bias chunk: replicate DRAM row across partitions (step-0 AP)
                nc.sync.dma_start(
                    out=bias_sb[:, nsl],
                    in_=bias[0:1, nsl].to_broadcast([P, N_TILE]),
                )
                for bt in range(B_TILES):
                    acc = ps.tile([P, N_TILE], F32, tag="ps")
                    for kb in range(K_TILES):
                        nc.tensor.matmul(
                            out=acc[:],
                            lhsT=m_T[bt][kb][:],
                            rhs=wsb[:, kb, :],
                            start=(kb == 0),
                            stop=(kb == K_TILES - 1),
                        )
                    osb = op.tile([P, N_TILE], F32, tag="o")
                    nc.vector.tensor_tensor(
                        out=osb[:],
                        in0=acc[:],
                        in1=bias_sb[:, nsl],
                        op=mybir.AluOpType.add,
                    )
                    nc.scalar.dma_start(
                        out=out[bt * P:(bt + 1) * P, n * N_TILE:n * N_TILE + cols],
                        in_=osb[:, :cols],
                    )
    nc.compile()
    return nc


_NC = None


def _get_nc():
    global _NC
    if _NC is None:
        _NC = build_nc()
    return _NC


def _np_to_fp32r(x: np.ndarray) -> np.ndarray:
    """Round fp32 -> fp32r encoding (IEEE bits with mantissa RTN-rounded to 11
    explicit bits, low 12 bits zero). Matches walrus cast_fp32_to_fp32r exactly.
    The PE consumes fp32r at full rate; raw IEEE bits in an fp32r operand are
    misinterpreted, so the encode must happen before upload."""
    u = np.ascontiguousarray(x, np.float32).view(np.uint32)
    keep = (u >> 12).astype(np.uint32)
    rbit = (u >> 11) & np.uint32(1)
    sticky = (u & np.uint32(0x7FF)) != 0
    inc = (rbit & (sticky.astype(np.uint32) | (keep & np.uint32(1)))).astype(np.uint32)
    return (((keep + inc) << np.uint32(12)).astype(np.uint32)).view(np.float32)


def _make_in_maps(batch_data, ent2word, words_embd, rel_embed, W, b):
    bd32 = np.ascontiguousarray(np.asarray(batch_data).astype(np.int32))
    e2w = np.ascontiguousarray(np.asarray(ent2word, dtype=np.int32))
    words = np.ascontiguousarray(np.asarray(words_embd, dtype=np.float32))
    rels = np.ascontiguousarray(np.asarray(rel_embed, dtype=np.float32))
    W = np.asarray(W, dtype=np.float32)
    b = np.asarray(b, dtype=np.float32)

    in_maps = []
    for c in range(N_CORES):
        sl = slice(c * E_SHARD, (c + 1) * E_SHARD)
        wt = np.zeros((K, E_PAD), np.float32)
        wt[:, :E_SHARD] = W[sl].T
        if COMPUTE_DTYPE == "f32r":
            wt = _np_to_fp32r(wt)
        else:
            wt = wt.astype(ml_dtypes.bfloat16)
        bias = np.zeros((1, E_PAD), np.float32)
        bias[0, :E_SHARD] = b[sl]
        in_maps.append(
            {"bd": bd32, "e2w": e2w, "words": words, "rels": rels,
             "wt": wt, "bias": bias}
        )
    return in_maps


def run(trace=False, **inputs):
    nc = _get_nc()
    in_maps = _make_in_maps(**inputs)
    res = run_bass_kernel_spmd(nc, in_maps, core_ids=list(range(N_CORES)), trace=trace)
    out = np.concatenate([r["out"] for r in res.results], axis=1)
    return out, res


def kernel(**inputs) -> np.ndarray:
    out, _ = run(trace=False, **inputs)
    return out
